# revision 1
# baseline (speedup 1.0000x reference)
"""GQA attention kernel for Trainium2, sharded over 8 NeuronCores.

Problem (hardcoded): B=4, S=1024, HID=2048, 16 query heads, 4 KV heads,
head_dim=128, RoPE (base 10000), causal softmax, O-projection.

Sharding: core c handles (batch b = c//2, head-half = c%2): 8 query heads,
2 KV heads, and the matching column/row shards of Wq/Wk/Wv/Wo. Each core
produces a partial O-projection output [S, HID]; the host sums the two
halves per batch element.

Per-core pipeline (all layouts chosen so no transposes are needed beyond
one x -> xT pass):
  xT = transpose(x)                      via PE transpose, [HID, S]
  qT_h = Wq_h^T @ x^T   (f32r matmuls)   [128d, S]  then RoPE via P64 perm
  kT_c = Wk_c^T @ x^T                    [128d, S]  then RoPE
  v    = x @ Wv          (natural)       [S, 256]   cast bf16
  scoresT[k,q] blocks = kT^T qT          block-causal only
  expS = exp(scoresT * 1/sqrt(128))      ACT, bf16, masked on diagonal blocks
  denom[q] = ones^T @ (sum_k expS)       DVE adds + PE ones-matmul
  recip = reciprocal_approx_fast(denom)
  ctxT_h = V^T @ expS                    bf16 matmul, then * recip (PE bcast)
  out = sum_h ctxT_h^T @ Wo_h            f32r matmuls
"""
import math
from contextlib import ExitStack

import numpy as np

import concourse.bass as bass
import concourse.bacc as bacc
import concourse.tile as tile
from concourse import mybir
from concourse.bass_utils import run_bass_kernel_spmd

F32 = mybir.dt.float32
F32R = mybir.dt.float32r
BF16 = mybir.dt.bfloat16

B, S, HID = 4, 1024, 2048
NH, NKV, D = 16, 4, 128
HPC = 8          # query heads per core
KVPC = 2         # kv heads per core
SCALE = 1.0 / math.sqrt(D)
NKT = S // 128   # 8 k-position tiles
NQC = S // 512   # 2 q chunks
NHT = HID // 128  # 16 hid k-tiles


def _valid_blocks():
    """(kt, j) score blocks under block-causality; k-tile kt=[128kt,128kt+128),
    q-chunk j=[512j, 512j+512). Valid iff some q >= k in block."""
    out = []
    for kt in range(NKT):
        for j in range(NQC):
            if 512 * j + 511 >= 128 * kt:
                out.append((kt, j))
    return out


VALID = _valid_blocks()          # 12 blocks
# straddling blocks need an elementwise mask with relative offset r=128*ri
def _mask_ri(kt, j):
    r = 128 * kt - 512 * j
    if 0 <= r < 512:
        return r // 128
    return None


def build_kernel():
    nc = bacc.Bacc(None)
    x = nc.dram_tensor("x", [S, HID], F32, kind="ExternalInput")
    wq = nc.dram_tensor("wq", [HID, HPC * D], F32, kind="ExternalInput")
    wk = nc.dram_tensor("wk", [HID, KVPC * D], F32, kind="ExternalInput")
    wv = nc.dram_tensor("wv", [HID, KVPC * D], F32, kind="ExternalInput")
    wo = nc.dram_tensor("wo", [HPC * D, HID], F32, kind="ExternalInput")
    cosT = nc.dram_tensor("cosT", [D, S], F32, kind="ExternalInput")
    sinTs = nc.dram_tensor("sinTs", [D, S], F32, kind="ExternalInput")
    p64 = nc.dram_tensor("p64", [D, D], F32, kind="ExternalInput")
    ident = nc.dram_tensor("ident", [128, 128], F32, kind="ExternalInput")
    masks = nc.dram_tensor("masks", [128, 4 * 512], F32, kind="ExternalInput")
    out = nc.dram_tensor("out", [S, HID], F32, kind="ExternalOutput")

    with tile.TileContext(nc) as tc, ExitStack() as top:
        # ---- long-lived pools -------------------------------------------
        const = top.enter_context(tc.tile_pool(name="const", bufs=1))
        kT_pool = top.enter_context(tc.tile_pool(name="kT", bufs=1))
        v_pool = top.enter_context(tc.tile_pool(name="v", bufs=1))
        ctxT_pool = top.enter_context(tc.tile_pool(name="ctxT", bufs=1))
        dn_pool = top.enter_context(tc.tile_pool(name="dn", bufs=2))
        psA = top.enter_context(tc.tile_pool(name="psA", bufs=2, space="PSUM"))
        psS = top.enter_context(tc.tile_pool(name="psS", bufs=2, space="PSUM"))
        psC = top.enter_context(tc.tile_pool(name="psC", bufs=1, space="PSUM"))

        # constants
        cosT_sb = const.tile([D, S], F32R)
        sinTs_sb = const.tile([D, S], F32R)
        p64_sb = const.tile([D, D], F32R)
        id_sb = const.tile([128, 128], F32)
        masks_sb = const.tile([128, 4 * 512], BF16)
        ones_bf = const.tile([128, 1], BF16)
        ones1p_f = const.tile([1, 128], F32)
        ones1p = const.tile([1, 128], F32R)
        nc.sync.dma_start(cosT_sb[:], cosT[:].bitcast(F32R))
        nc.sync.dma_start(sinTs_sb[:], sinTs[:].bitcast(F32R))
        nc.sync.dma_start(p64_sb[:], p64[:].bitcast(F32R))
        nc.sync.dma_start(id_sb[:], ident[:])
        # masks arrive f32; cast to bf16 on DVE via a staging tile
        with tc.tile_pool(name="mstage", bufs=1) as mstage:
            mtmp = mstage.tile([128, 4 * 512], F32)
            nc.sync.dma_start(mtmp[:], masks[:])
            nc.vector.tensor_copy(masks_sb[:], mtmp[:])
        nc.vector.memset(ones_bf[:], 1.0)
        nc.vector.memset(ones1p_f[:], 1.0)
        nc.vector.tensor_copy(ones1p[:], ones1p_f[:])

        # per-head f32 ctxT tiles (normalized), resident until O-proj
        ctxT = [ctxT_pool.tile([D, S], F32R, tag=f"ctxT{h}", name=f"ctxT{h}")
                for h in range(HPC)]

        with ExitStack() as ph1:
            xT_pool = ph1.enter_context(tc.tile_pool(name="xT", bufs=1))
            tmp_pool = ph1.enter_context(tc.tile_pool(name="tmp", bufs=2))

            # ---- x transpose: xT[128, kt*1024 + s] ----------------------
            xT = xT_pool.tile([128, NHT * S], F32R)
            xin_ctx = ExitStack()
            xin_pool = xin_ctx.enter_context(tc.tile_pool(name="xin", bufs=2))
            for st in range(S // 128):
                xtile = xin_pool.tile([128, HID], F32)
                nc.sync.dma_start(xtile[:], x[st * 128:(st + 1) * 128, :])
                for ht in range(NHT):
                    tp = psA.tile([128, 512], F32, tag="ps_a")
                    nc.tensor.transpose(
                        tp[:, 0:128], xtile[:, ht * 128:(ht + 1) * 128], id_sb[:])
                    nc.vector.tensor_copy(
                        xT[:, ht * S + st * 128: ht * S + (st + 1) * 128],
                        tp[:, 0:128])

            xin_ctx.close()

            # ---- K/V weights ([p, kt*KVD + d] layout, scoped pool) ------
            KVD = KVPC * D  # 256
            wkv_ctx = ExitStack()
            wkv_pool = wkv_ctx.enter_context(tc.tile_pool(name="wkv", bufs=1))
            wk_sb = wkv_pool.tile([128, NHT * KVD], F32R, tag="wk")
            wv_sb = wkv_pool.tile([128, NHT * KVD], F32R, tag="wv")
            nc.sync.dma_start(
                wk_sb[:].rearrange("p (kt d) -> p kt d", kt=NHT),
                wk[:].bitcast(F32R).rearrange("(kt p) d -> p kt d", p=128))
            nc.sync.dma_start(
                wv_sb[:].rearrange("p (kt d) -> p kt d", kt=NHT),
                wv[:].bitcast(F32R).rearrange("(kt p) d -> p kt d", p=128))

            def rope_T(dst_sb, src_sb, sc):
                """RoPE in T orientation on 512-col chunk sc of a [128, S] tile.
                dst/src may alias. Uses P64 perm matmul + 3 DVE TT ops."""
                cs = slice(sc * 512, (sc + 1) * 512)
                rot = psA.tile([128, 512], F32, tag="ps_a")
                nc.tensor.matmul(rot[:], p64_sb[:], src_sb[:, cs],
                                 start=True, stop=True)
                tmp = tmp_pool.tile([128, 512], F32R, tag="ropetmp")
                nc.vector.tensor_mul(tmp[:], rot[:].bitcast(F32R), sinTs_sb[:, cs])
                nc.vector.tensor_mul(dst_sb[:, cs], src_sb[:, cs], cosT_sb[:, cs])
                nc.vector.tensor_add(dst_sb[:, cs], dst_sb[:, cs], tmp[:])

            # ---- K projection + rope: kT_c [128d, S] --------------------
            kT = [kT_pool.tile([D, S], F32R, tag=f"kT{c}", name=f"kT{c}")
                  for c in range(KVPC)]
            for c in range(KVPC):
                for sc in range(NQC):
                    ps = psA.tile([128, 512], F32, tag="ps_a")
                    for kt in range(NHT):
                        nc.tensor.matmul(
                            ps[:],
                            wk_sb[:, kt * KVD + c * D:kt * KVD + (c + 1) * D],
                            xT[:, kt * S + sc * 512:kt * S + (sc + 1) * 512],
                            start=(kt == 0), stop=(kt == NHT - 1))
                    nc.vector.tensor_copy(kT[c][:, sc * 512:(sc + 1) * 512],
                                          ps[:].bitcast(F32R))
                for sc in range(NQC):
                    rope_T(kT[c], kT[c], sc)

            # ---- V projection (natural): v [s, kv*128 + d] bf16 ---------
            v_sb = v_pool.tile([128, (S // 128) * KVD], BF16)
            for st in range(S // 128):
                ps = psA.tile([128, 512], F32, tag="ps_a")
                for kt in range(NHT):
                    nc.tensor.matmul(
                        ps[:, 0:KVD],
                        xT[:, kt * S + st * 128:kt * S + (st + 1) * 128],
                        wv_sb[:, kt * KVD:(kt + 1) * KVD],
                        start=(kt == 0), stop=(kt == NHT - 1))
                nc.vector.tensor_copy(v_sb[:, st * KVD:(st + 1) * KVD],
                                      ps[:, 0:KVD])

            wkv_ctx.close()
            wq_pool = ph1.enter_context(tc.tile_pool(name="wq", bufs=2))
            qT_pool = ph1.enter_context(tc.tile_pool(name="qT", bufs=2))
            eS_pool = ph1.enter_context(tc.tile_pool(name="eS", bufs=2))
            rc_pool = ph1.enter_context(tc.tile_pool(name="rc", bufs=1))

            # ---- per-head: Q proj + rope + attention --------------------
            for h in range(HPC):
                c = h // (HPC // KVPC)  # local kv head
                # Q weights for this head: [p, kt*128 + d]
                wq_h = wq_pool.tile([128, NHT * D], F32R, tag="wqh")
                nc.sync.dma_start(
                    wq_h[:].rearrange("p (kt d) -> p kt d", kt=NHT),
                    wq[:, h * D:(h + 1) * D].bitcast(F32R)
                    .rearrange("(kt p) d -> p kt d", p=128))
                qT_h = qT_pool.tile([D, S], F32R, tag="qTh")
                for sc in range(NQC):
                    ps = psA.tile([128, 512], F32, tag="ps_a")
                    for kt in range(NHT):
                        nc.tensor.matmul(
                            ps[:],
                            wq_h[:, kt * D:(kt + 1) * D],
                            xT[:, kt * S + sc * 512:kt * S + (sc + 1) * 512],
                            start=(kt == 0), stop=(kt == NHT - 1))
                    nc.vector.tensor_copy(qT_h[:, sc * 512:(sc + 1) * 512],
                                          ps[:].bitcast(F32R))
                for sc in range(NQC):
                    rope_T(qT_h, qT_h, sc)

                # scores + exp + mask; expS packed [128, 12*512] bf16
                eS = eS_pool.tile([128, len(VALID) * 512], BF16, tag="eS")
                off = {}
                o = 0
                for kt, j in VALID:
                    off[(kt, j)] = o
                    o += 512
                for kt in range(NKT):
                    js = [j for (k2, j) in VALID if k2 == kt]
                    w = len(js) * 512
                    ps = psS.tile([128, 1024], F32, tag="ps_s")
                    for i, j in enumerate(js):
                        nc.tensor.matmul(
                            ps[:, i * 512:(i + 1) * 512],
                            kT[c][:, kt * 128:(kt + 1) * 128],
                            qT_h[:, j * 512:(j + 1) * 512],
                            start=True, stop=True)
                    # exp over the packed [128, w] block in one ACT op
                    o0 = off[(kt, js[0])]
                    nc.scalar.activation(
                        eS[:, o0:o0 + w], ps[:, 0:w],
                        mybir.ActivationFunctionType.Exp, scale=SCALE)
                    for j in js:
                        ri = _mask_ri(kt, j)
                        if ri is not None:
                            oj = off[(kt, j)]
                            nc.vector.tensor_mul(
                                eS[:, oj:oj + 512], eS[:, oj:oj + 512],
                                masks_sb[:, ri * 512:(ri + 1) * 512])

                # denominator partials (bf16 adds over k-tiles)
                dnp = dn_pool.tile([128, S], BF16, tag="dnp")
                for j in range(NQC):
                    kts = [kt for (kt, j2) in VALID if j2 == j]
                    js = slice(j * 512, (j + 1) * 512)
                    nc.vector.tensor_add(
                        dnp[:, js],
                        eS[:, off[(kts[0], j)]:off[(kts[0], j)] + 512],
                        eS[:, off[(kts[1], j)]:off[(kts[1], j)] + 512])
                    for kt in kts[2:]:
                        nc.vector.tensor_add(
                            dnp[:, js], dnp[:, js],
                            eS[:, off[(kt, j)]:off[(kt, j)] + 512])
                # ones-matmul -> denom [1, 512] per chunk; recip
                rc = rc_pool.tile([1, S], F32, tag="rc")
                rc_r = rc_pool.tile([1, S], F32R, tag="rcr")
                for j in range(NQC):
                    dps = psA.tile([128, 512], F32, tag="ps_a")
                    nc.tensor.matmul(dps[:1, 0:512], ones_bf[:],
                                     dnp[:, j * 512:(j + 1) * 512],
                                     start=True, stop=True)
                    nc.vector.reciprocal_approx_fast(
                        rc[:1, j * 512:(j + 1) * 512], dps[:1, 0:512])
                    nc.vector.tensor_copy(
                        rc_r[:1, j * 512:(j + 1) * 512],
                        rc[:1, j * 512:(j + 1) * 512])

                # ctx matmul (bf16) + normalize into ctxT
                pc = psC.tile([128, 1024], F32, tag="ps_c")
                for j in range(NQC):
                    kts = [kt for (kt, j2) in VALID if j2 == j]
                    for i, kt in enumerate(kts):
                        nc.tensor.matmul(
                            pc[:, j * 512:(j + 1) * 512],
                            v_sb[:, kt * KVD + c * D:kt * KVD + (c + 1) * D],
                            eS[:, off[(kt, j)]:off[(kt, j)] + 512],
                            start=(i == 0), stop=(i == len(kts) - 1))
                for j in range(NQC):
                    bc = psA.tile([128, 512], F32, tag="ps_a")
                    nc.tensor.matmul(
                        bc[:], ones1p[:],
                        rc_r[:1, j * 512:(j + 1) * 512],
                        start=True, stop=True)
                    rb = tmp_pool.tile([128, 512], F32R, tag="rbtmp")
                    nc.vector.tensor_copy(rb[:], bc[:].bitcast(F32R))
                    nc.vector.tensor_mul(
                        ctxT[h][:, j * 512:(j + 1) * 512],
                        pc[:, j * 512:(j + 1) * 512].bitcast(F32R),
                        rb[:])

        # ---- O projection ----------------------------------------------
        with ExitStack() as ph2:
            wo_pool = ph2.enter_context(tc.tile_pool(name="wo", bufs=1))
            out_pool = ph2.enter_context(tc.tile_pool(name="outp", bufs=3))
            wo_sb = [wo_pool.tile([128, HID], F32R, tag=f"wo{h}", name=f"wo{h}")
                     for h in range(HPC)]
            for h in range(HPC):
                nc.sync.dma_start(wo_sb[h][:],
                                  wo[h * D:(h + 1) * D, :].bitcast(F32R))
            for pt in range(S // 128):
                for ec in range(HID // 512):
                    po = psA.tile([128, 512], F32, tag="ps_a")
                    for h in range(HPC):
                        nc.tensor.matmul(
                            po[:],
                            ctxT[h][:, pt * 128:(pt + 1) * 128],
                            wo_sb[h][:, ec * 512:(ec + 1) * 512],
                            start=(h == 0), stop=(h == HPC - 1))
                    ot = out_pool.tile([128, 512], F32, tag="ot")
                    nc.vector.tensor_copy(ot[:], po[:])
                    nc.sync.dma_start(
                        out[pt * 128:(pt + 1) * 128, ec * 512:(ec + 1) * 512],
                        ot[:])
    nc.finalize()
    return nc


def host_tables():
    inv_freq = 1.0 / (10000.0 ** (np.arange(0, D, 2, dtype=np.float64) / D))
    t = np.arange(S, dtype=np.float64)
    freqs = np.outer(t, inv_freq)
    emb = np.concatenate([freqs, freqs], -1)
    cosT = np.cos(emb).T.astype(np.float32).copy()
    sinTs = np.sin(emb).T.astype(np.float32).copy()
    sinTs[:64] *= -1.0
    p64 = np.zeros((D, D), dtype=np.float32)
    for d in range(D):
        p64[d, (d + 64) % D] = 1.0
    ident = np.eye(128, dtype=np.float32)
    masks = np.zeros((128, 4 * 512), dtype=np.float32)
    for ri in range(4):
        r = 128 * ri
        for p in range(128):
            c0 = p + r
            if c0 < 512:
                masks[p, ri * 512 + c0:(ri + 1) * 512] = 1.0
    return cosT, sinTs, p64, ident, masks


_CACHE = {}


def kernel(hidden_states, Wq, Wk, Wv, Wo, _trace=False, _tmpdir=None):
    hidden_states = np.ascontiguousarray(hidden_states, dtype=np.float32)
    Wq = np.ascontiguousarray(Wq, dtype=np.float32)
    Wk = np.ascontiguousarray(Wk, dtype=np.float32)
    Wv = np.ascontiguousarray(Wv, dtype=np.float32)
    Wo = np.ascontiguousarray(Wo, dtype=np.float32)

    if "nc" not in _CACHE:
        _CACHE["nc"] = build_kernel()
    nc = _CACHE["nc"]
    cosT, sinTs, p64, ident, masks = host_tables()

    in_maps = []
    for c in range(8):
        b, half = c // 2, c % 2
        in_maps.append({
            "x": hidden_states[b],
            "wq": np.ascontiguousarray(Wq[:, half * 1024:(half + 1) * 1024]),
            "wk": np.ascontiguousarray(Wk[:, half * 256:(half + 1) * 256]),
            "wv": np.ascontiguousarray(Wv[:, half * 256:(half + 1) * 256]),
            "wo": np.ascontiguousarray(Wo[half * 1024:(half + 1) * 1024, :]),
            "cosT": cosT, "sinTs": sinTs, "p64": p64, "ident": ident,
            "masks": masks,
        })
    res = run_bass_kernel_spmd(nc, in_maps, list(range(8)),
                               trace=_trace, tmpdir=_tmpdir)
    out = np.zeros((B, S, HID), dtype=np.float32)
    for c in range(8):
        out[c // 2] += res.results[c]["out"]
    if _trace:
        return out, res
    return out



# revision 11
# speedup vs baseline: 1.8201x; 1.8201x over previous
"""GQA attention kernel for Trainium2, sharded over 8 NeuronCores.

Problem (hardcoded): B=4, S=1024, HID=2048, 16 query heads, 4 KV heads,
head_dim=128, RoPE (base 10000), causal softmax, O-projection.

Sharding: core c handles (batch b = c//2, head-half = c%2): 8 query heads,
2 KV heads, and the matching column/row shards of Wq/Wk/Wv/Wo. Each core
produces a partial O-projection output [S, HID]; the host sums the two
halves per batch element.

v2: all-bf16 datapath (PE runs 1 cycle/row with FWL weight loads, DMA bytes
halved), host-side transpose/relayout of x and weights (no on-device
transposes), ragged block-causal score/ctx blocks at 128-row granularity
(25% less attention work than 512-wide blocks), and a software-pipelined
per-head schedule (Q-proj of head h+1 issued between scores(h) and ctx(h))
so the tensor engine never idles long enough for HAM to re-throttle it.

Per-core layouts (partition dim first, 128 everywhere):
  xT   [128, kt(16), s(1024)]   xT[p,kt,s] = x[s, 128kt+p]          bf16
  wq   [128, h(8), kt(16), d(128)]                                  bf16
  wk/wv[128, kt(16), c(2)*d(128)]                                   bf16
  wo   [128, h(8), e(2048)]     wo[p,h,e] = Wo[1024*half+128h+p, e] bf16
  qT/kT[128d, s]   roped in T orientation via P64 perm matmul
  v    [128s, st(8)*c(2)*d(128)]  natural, for ctx stationary
  eS   [128k, ragged q spans]   exp(scores^T) bf16
  ctxT [128d, s] per head       normalized context, feeds O-proj
"""
import math
from contextlib import ExitStack

import numpy as np
import ml_dtypes

import concourse.bass as bass
import concourse.bacc as bacc
import concourse.tile as tile
from concourse import mybir
from concourse.bass_utils import run_bass_kernel_spmd

F32 = mybir.dt.float32
F32R = mybir.dt.float32r
BF16 = mybir.dt.bfloat16

B, S, HID = 4, 1024, 2048
NH, NKV, D = 16, 4, 128
HPC = 8          # query heads per core
KVPC = 2         # kv heads per core
SCALE = 1.0 / math.sqrt(D)
NKT = HID // 128  # 16 contraction tiles
NST = S // 128    # 8 sequence tiles
KVD = KVPC * D    # 256

# score block spans: for k-tile kt, valid q span is [128*kt, S), split into
# PSUM-bank-sized pieces aligned to 512 boundaries.
def _pieces(kt):
    qlo = 128 * kt
    out = []
    while qlo < S:
        qhi = min(S, (qlo // 512 + 1) * 512)
        out.append((qlo, qhi))
        qlo = qhi
    return out

# eS packing offset per kt (ragged)
ES_OFF = []
_o = 0
for _kt in range(NST):
    ES_OFF.append(_o)
    _o += S - 128 * _kt
ES_W = _o  # 4608


def build_kernel():
    nc = bacc.Bacc(None)
    xT = nc.dram_tensor("xT", [128, NKT * S], BF16, kind="ExternalInput")
    wq = nc.dram_tensor("wq", [128, HPC * NKT * D], BF16, kind="ExternalInput")
    wk = nc.dram_tensor("wk", [128, NKT * KVD], BF16, kind="ExternalInput")
    wv = nc.dram_tensor("wv", [128, NKT * KVD], BF16, kind="ExternalInput")
    wo = nc.dram_tensor("wo", [128, HPC * HID], BF16, kind="ExternalInput")
    cosT = nc.dram_tensor("cosT", [D, S], BF16, kind="ExternalInput")
    sinTs = nc.dram_tensor("sinTs", [D, S], BF16, kind="ExternalInput")
    p64 = nc.dram_tensor("p64", [D, D], BF16, kind="ExternalInput")
    dmask = nc.dram_tensor("dmask", [128, 128], BF16, kind="ExternalInput")
    out = nc.dram_tensor("out", [S, HID], F32, kind="ExternalOutput")

    with tile.TileContext(nc) as tc, ExitStack() as top:
        # ---- pools ------------------------------------------------------
        const = top.enter_context(tc.tile_pool(name="const", bufs=1))
        xk_pool = top.enter_context(tc.tile_pool(name="xk", bufs=1))
        wkv_pool = top.enter_context(tc.tile_pool(name="wkv", bufs=1))
        kT_pool = top.enter_context(tc.tile_pool(name="kT", bufs=1))
        v_pool = top.enter_context(tc.tile_pool(name="v", bufs=1))
        ctxT_pool = top.enter_context(tc.tile_pool(name="ctxT", bufs=1))
        wq_pool = top.enter_context(tc.tile_pool(name="wq", bufs=2))
        qT_pool = top.enter_context(tc.tile_pool(name="qT", bufs=2))
        eS_pool = top.enter_context(tc.tile_pool(name="eS", bufs=2))
        dn_pool = top.enter_context(tc.tile_pool(name="dn", bufs=2))
        rc_pool = top.enter_context(tc.tile_pool(name="rc", bufs=2))
        tmp_pool = top.enter_context(tc.tile_pool(name="tmp", bufs=2))
        wo_pool = top.enter_context(tc.tile_pool(name="wo", bufs=1))
        out_pool = top.enter_context(tc.tile_pool(name="outp", bufs=3))
        psQ = top.enter_context(tc.tile_pool(name="psQ", bufs=2, space="PSUM"))
        psS = top.enter_context(tc.tile_pool(name="psS", bufs=2, space="PSUM"))
        psC = top.enter_context(tc.tile_pool(name="psC", bufs=2, space="PSUM"))
        psD = top.enter_context(tc.tile_pool(name="psD", bufs=2, space="PSUM"))

        # ---- constants & input DMA (issue order = priority) -------------
        wk_sb = wkv_pool.tile([128, NKT * KVD], BF16, tag="wk")
        wv_sb = wkv_pool.tile([128, NKT * KVD], BF16, tag="wv")
        nc.sync.dma_start(wk_sb[:], wk[:])
        nc.sync.dma_start(wv_sb[:], wv[:])

        cosT_sb = const.tile([D, S], BF16)
        sinTs_sb = const.tile([D, S], BF16)
        p64_sb = const.tile([D, D], BF16)
        dmask_sb = const.tile([128, 128], BF16)
        ones_bf = const.tile([128, 1], BF16)
        ones1p_f = const.tile([1, 128], F32)
        ones1p = const.tile([1, 128], F32R)
        nc.sync.dma_start(cosT_sb[:], cosT[:])
        nc.sync.dma_start(sinTs_sb[:], sinTs[:])
        nc.sync.dma_start(p64_sb[:], p64[:])
        nc.sync.dma_start(dmask_sb[:], dmask[:])
        nc.vector.memset(ones_bf[:], 1.0)
        nc.vector.memset(ones1p_f[:], 1.0)
        nc.vector.tensor_copy(ones1p[:], ones1p_f[:])

        # x^T arrives in 16 kt-chunks so projections can chase the DMA
        xk = [xk_pool.tile([128, S], BF16, tag=f"xk{kt}", name=f"xk{kt}")
              for kt in range(NKT)]
        for kt in range(NKT):
            nc.sync.dma_start(xk[kt][:], xT[:, kt * S:(kt + 1) * S])

        wq_sb = [wq_pool.tile([128, NKT * D], BF16, tag="wqh",
                              name=f"wqh{h}") for h in range(HPC)]
        for h in range(2):
            nc.sync.dma_start(wq_sb[h][:],
                              wq[:, h * NKT * D:(h + 1) * NKT * D])

        ctxT = [ctxT_pool.tile([D, S], BF16, tag=f"ctxT{h}", name=f"ctxT{h}")
                for h in range(HPC)]

        def rope_T(dst_sb, sc):
            """RoPE in T orientation on 512-col chunk sc of a [128, S] bf16
            tile, in place. P64 perm matmul + 3 DVE ops."""
            cs = slice(sc * 512, (sc + 1) * 512)
            rot = psD.tile([128, 512], F32, tag="ps_d")
            nc.tensor.matmul(rot[:], p64_sb[:], dst_sb[:, cs],
                             start=True, stop=True)
            tmp = tmp_pool.tile([128, 512], BF16, tag="ropetmp")
            nc.vector.tensor_mul(tmp[:], rot[:], sinTs_sb[:, cs])
            nc.vector.tensor_mul(dst_sb[:, cs], dst_sb[:, cs], cosT_sb[:, cs])
            nc.vector.tensor_add(dst_sb[:, cs], dst_sb[:, cs], tmp[:])

        # ---- K projection + rope: kT_c [128d, S] ------------------------
        kT = [kT_pool.tile([D, S], BF16, tag=f"kT{c}", name=f"kT{c}")
              for c in range(KVPC)]
        for c in range(KVPC):
            for sc in range(2):
                ps = psQ.tile([128, 512], F32, tag="ps_q")
                for kt in range(NKT):
                    nc.tensor.matmul(
                        ps[:], wk_sb[:, kt * KVD + c * D:kt * KVD + (c + 1) * D],
                        xk[kt][:, sc * 512:(sc + 1) * 512],
                        start=(kt == 0), stop=(kt == NKT - 1))
                nc.vector.tensor_copy(kT[c][:, sc * 512:(sc + 1) * 512], ps[:])
            for sc in range(2):
                rope_T(kT[c], sc)

        # ---- V projection (natural): v [128s, st*KVD + c*D + d] ---------
        v_sb = v_pool.tile([128, NST * KVD], BF16)
        for st in range(NST):
            ps = psQ.tile([128, 512], F32, tag="ps_q")
            for kt in range(NKT):
                nc.tensor.matmul(
                    ps[:, 0:KVD], xk[kt][:, st * 128:(st + 1) * 128],
                    wv_sb[:, kt * KVD:(kt + 1) * KVD],
                    start=(kt == 0), stop=(kt == NKT - 1))
            nc.vector.tensor_copy(v_sb[:, st * KVD:(st + 1) * KVD],
                                  ps[:, 0:KVD])

        # ---- per-head pipeline ------------------------------------------
        def q_proj(h):
            qT_h = qT_pool.tile([D, S], BF16, tag="qTh", name=f"qT{h}")
            for sc in range(2):
                ps = psQ.tile([128, 512], F32, tag="ps_q")
                for kt in range(NKT):
                    nc.tensor.matmul(
                        ps[:], wq_sb[h][:, kt * D:(kt + 1) * D],
                        xk[kt][:, sc * 512:(sc + 1) * 512],
                        start=(kt == 0), stop=(kt == NKT - 1))
                # PSUM->SBUF cast on the scalar engine (DVE is busier)
                nc.scalar.copy(qT_h[:, sc * 512:(sc + 1) * 512], ps[:])
            for sc in range(2):
                rope_T(qT_h, sc)
            return qT_h

        qT_cur = q_proj(0)

        wo_sb = None
        for h in range(HPC):
            c = h // (HPC // KVPC)  # local kv head
            # -- scores + exp (ragged blocks per kt) ----------------------
            eS = eS_pool.tile([128, ES_W], BF16, tag="eS", name=f"eS{h}")
            for kt in range(NST):
                for (qlo, qhi) in _pieces(kt):
                    w = qhi - qlo
                    ps = psS.tile([128, 512], F32, tag="ps_s")
                    nc.tensor.matmul(
                        ps[:, 0:w], kT[c][:, kt * 128:(kt + 1) * 128],
                        qT_cur[:, qlo:qhi], start=True, stop=True)
                    nc.scalar.activation(
                        eS[:, ES_OFF[kt] + qlo - 128 * kt:
                            ES_OFF[kt] + qhi - 128 * kt],
                        ps[:, 0:w],
                        mybir.ActivationFunctionType.Exp, scale=SCALE)
                # mask the diagonal 128x128 block
                nc.vector.tensor_mul(
                    eS[:, ES_OFF[kt]:ES_OFF[kt] + 128],
                    eS[:, ES_OFF[kt]:ES_OFF[kt] + 128], dmask_sb[:])

            # -- denominator ragged pre-sum, issued BEFORE the next head's
            # rope so it trails exp(h) while the PE runs Q-proj.  j=0 on
            # DVE, j=1 on the otherwise-idle gpsimd engine.
            dnp = dn_pool.tile([128, S], BF16, tag="dnp", name=f"dnp{h}")
            for j, eng in ((0, nc.vector), (1, nc.gpsimd)):
                first = True
                for kt in range(NST):
                    qlo = 128 * kt
                    lo = max(qlo, j * 512)
                    hi = min(S, (j + 1) * 512)
                    if lo >= hi:
                        continue
                    src = eS[:, ES_OFF[kt] + lo - qlo:ES_OFF[kt] + hi - qlo]
                    if first:
                        eng.tensor_copy(dnp[:, lo:hi], src)
                        first = False
                    else:
                        eng.tensor_add(dnp[:, lo:hi], dnp[:, lo:hi], src)

            # -- next head's Q proj fills the PE while exp(h) runs --------
            if h + 1 < HPC:
                if h + 2 < HPC:
                    nc.sync.dma_start(
                        wq_sb[h + 2][:],
                        wq[:, (h + 2) * NKT * D:(h + 3) * NKT * D])
                qT_nxt = q_proj(h + 1)

            # -- denominator ones-matmul + reciprocal ---------------------
            rc = rc_pool.tile([1, S], F32, tag="rc", name=f"rc{h}")
            rc_r = rc_pool.tile([1, S], F32R, tag="rcr", name=f"rcr{h}")
            dps = [None, None]
            for j in range(2):
                dps[j] = psD.tile([128, 512], F32, tag="ps_d", name=f"dps{j}")
                nc.tensor.matmul(dps[j][:1, 0:512], ones_bf[:],
                                 dnp[:, j * 512:(j + 1) * 512],
                                 start=True, stop=True)
            for j in range(2):
                nc.vector.reciprocal_approx_fast(
                    rc[:1, j * 512:(j + 1) * 512], dps[j][:1, 0:512])
                nc.vector.tensor_copy(rc_r[:1, j * 512:(j + 1) * 512],
                                      rc[:1, j * 512:(j + 1) * 512])

            # -- ctx matmul (ragged accumulate) + normalize ---------------
            pc = [None, None]
            for j in range(2):
                kts = [kt for kt in range(NST)
                       if max(128 * kt, j * 512) < (j + 1) * 512]
                pc[j] = psC.tile([128, 512], F32, tag="ps_c", name=f"pc{j}")
                for kt in kts:
                    qlo = 128 * kt
                    lo = max(qlo, j * 512)
                    hi = (j + 1) * 512
                    nc.tensor.matmul(
                        pc[j][:, lo - j * 512:hi - j * 512],
                        v_sb[:, kt * KVD + c * D:kt * KVD + (c + 1) * D],
                        eS[:, ES_OFF[kt] + lo - qlo:ES_OFF[kt] + hi - qlo],
                        start=(kt == kts[0]), stop=(kt == kts[-1]))
            for j in range(2):
                bc = psD.tile([128, 512], F32, tag="ps_d")
                nc.tensor.matmul(bc[:], ones1p[:],
                                 rc_r[:1, j * 512:(j + 1) * 512],
                                 start=True, stop=True)
                rb = tmp_pool.tile([128, 512], F32, tag="rbtmp")
                nc.scalar.copy(rb[:], bc[:])
                nc.vector.tensor_mul(
                    ctxT[h][:, j * 512:(j + 1) * 512], pc[j][:], rb[:])

            if h + 1 < HPC:
                qT_cur = qT_nxt
            if h == 3:  # wo arrives while attention still running
                wo_sb = wo_pool.tile([128, HPC * HID], BF16)
                nc.sync.dma_start(wo_sb[:], wo[:])

        # ---- O projection ----------------------------------------------
        for st in range(NST):
            for ec in range(HID // 512):
                if (st * 4 + ec) % 2:
                    po = psS.tile([128, 512], F32, tag="ps_s", name=f"po{st}_{ec}")
                else:
                    po = psQ.tile([128, 512], F32, tag="ps_q", name=f"po{st}_{ec}")
                for h in range(HPC):
                    nc.tensor.matmul(
                        po[:], ctxT[h][:, st * 128:(st + 1) * 128],
                        wo_sb[:, h * HID + ec * 512:h * HID + (ec + 1) * 512],
                        start=(h == 0), stop=(h == HPC - 1))
                ot = out_pool.tile([128, 512], F32, tag="ot")
                nc.vector.tensor_copy(ot[:], po[:])
                nc.sync.dma_start(
                    out[st * 128:(st + 1) * 128, ec * 512:(ec + 1) * 512],
                    ot[:])
    nc.finalize()
    return nc


def host_prep(hidden_states, Wq, Wk, Wv, Wo):
    """Pre-transpose/cast/relayout all inputs on the host (bf16)."""
    bf = ml_dtypes.bfloat16
    xTs = []
    for b in range(B):
        t = hidden_states[b].T.reshape(NKT, 128, S).transpose(1, 0, 2)
        xTs.append(np.ascontiguousarray(t.astype(bf)).reshape(128, NKT * S))
    halves = []
    for hf in range(2):
        wqh = Wq[:, 1024 * hf:1024 * (hf + 1)].reshape(NKT, 128, HPC, D)
        wqh = np.ascontiguousarray(
            wqh.transpose(1, 2, 0, 3).astype(bf)).reshape(128, HPC * NKT * D)
        wkh = Wk[:, KVD * hf:KVD * (hf + 1)].reshape(NKT, 128, KVD)
        wkh = np.ascontiguousarray(
            wkh.transpose(1, 0, 2).astype(bf)).reshape(128, NKT * KVD)
        wvh = Wv[:, KVD * hf:KVD * (hf + 1)].reshape(NKT, 128, KVD)
        wvh = np.ascontiguousarray(
            wvh.transpose(1, 0, 2).astype(bf)).reshape(128, NKT * KVD)
        woh = Wo[1024 * hf:1024 * (hf + 1), :].reshape(HPC, 128, HID)
        woh = np.ascontiguousarray(
            woh.transpose(1, 0, 2).astype(bf)).reshape(128, HPC * HID)
        halves.append((wqh, wkh, wvh, woh))

    inv_freq = 1.0 / (10000.0 ** (np.arange(0, D, 2, dtype=np.float64) / D))
    t = np.arange(S, dtype=np.float64)
    freqs = np.outer(t, inv_freq)
    emb = np.concatenate([freqs, freqs], -1)
    cosT = np.ascontiguousarray(np.cos(emb).T).astype(bf)
    sinTs_f = np.sin(emb).T.copy()
    sinTs_f[:64] *= -1.0
    sinTs = np.ascontiguousarray(sinTs_f).astype(bf)
    p64 = np.zeros((D, D), dtype=np.float32)
    for d in range(D):
        p64[d, (d + 64) % D] = 1.0
    p64 = p64.astype(bf)
    dmask = np.triu(np.ones((128, 128), dtype=np.float32)).astype(bf)
    return xTs, halves, cosT, sinTs, p64, dmask


_CACHE = {}


def kernel(hidden_states, Wq, Wk, Wv, Wo, _trace=False, _tmpdir=None):
    hidden_states = np.ascontiguousarray(hidden_states, dtype=np.float32)
    Wq = np.ascontiguousarray(Wq, dtype=np.float32)
    Wk = np.ascontiguousarray(Wk, dtype=np.float32)
    Wv = np.ascontiguousarray(Wv, dtype=np.float32)
    Wo = np.ascontiguousarray(Wo, dtype=np.float32)

    if "nc" not in _CACHE:
        _CACHE["nc"] = build_kernel()
    nc = _CACHE["nc"]
    xTs, halves, cosT, sinTs, p64, dmask = host_prep(
        hidden_states, Wq, Wk, Wv, Wo)

    in_maps = []
    for cid in range(8):
        b, hf = cid // 2, cid % 2
        wqh, wkh, wvh, woh = halves[hf]
        in_maps.append({
            "xT": xTs[b], "wq": wqh, "wk": wkh, "wv": wvh, "wo": woh,
            "cosT": cosT, "sinTs": sinTs, "p64": p64, "dmask": dmask,
        })
    res = run_bass_kernel_spmd(nc, in_maps, list(range(8)),
                               trace=_trace, tmpdir=_tmpdir)
    out = np.zeros((B, S, HID), dtype=np.float32)
    for cid in range(8):
        out[cid // 2] += res.results[cid]["out"]
    if _trace:
        return out, res
    return out


# revision 18
# speedup vs baseline: 1.8205x; 1.0002x over previous
"""GQA attention kernel for Trainium2, sharded over 8 NeuronCores.

Problem (hardcoded): B=4, S=1024, HID=2048, 16 query heads, 4 KV heads,
head_dim=128, RoPE (base 10000), causal softmax, O-projection.

Sharding: core c handles (batch b = c//2, head-half = c%2): 8 query heads,
2 KV heads, and the matching column/row shards of Wq/Wk/Wv/Wo. Each core
produces a partial O-projection output [S, HID]; the host sums the two
halves per batch element.

v2: all-bf16 datapath (PE runs 1 cycle/row with FWL weight loads, DMA bytes
halved), host-side transpose/relayout of x and weights (no on-device
transposes), ragged block-causal score/ctx blocks at 128-row granularity
(25% less attention work than 512-wide blocks), and a software-pipelined
per-head schedule (Q-proj of head h+1 issued between scores(h) and ctx(h))
so the tensor engine never idles long enough for HAM to re-throttle it.

Per-core layouts (partition dim first, 128 everywhere):
  xT   [128, kt(16), s(1024)]   xT[p,kt,s] = x[s, 128kt+p]          bf16
  wq   [128, h(8), kt(16), d(128)]                                  bf16
  wk/wv[128, kt(16), c(2)*d(128)]                                   bf16
  wo   [128, h(8), e(2048)]     wo[p,h,e] = Wo[1024*half+128h+p, e] bf16
  qT/kT[128d, s]   roped in T orientation via P64 perm matmul
  v    [128s, st(8)*c(2)*d(128)]  natural, for ctx stationary
  eS   [128k, ragged q spans]   exp(scores^T) bf16
  ctxT [128d, s] per head       normalized context, feeds O-proj
"""
import math
from contextlib import ExitStack

import numpy as np
import ml_dtypes

import concourse.bass as bass
import concourse.bacc as bacc
import concourse.tile as tile
from concourse import mybir
from concourse.bass_utils import run_bass_kernel_spmd

F32 = mybir.dt.float32
F32R = mybir.dt.float32r
BF16 = mybir.dt.bfloat16

B, S, HID = 4, 1024, 2048
NH, NKV, D = 16, 4, 128
HPC = 8          # query heads per core
KVPC = 2         # kv heads per core
SCALE = 1.0 / math.sqrt(D)
NKT = HID // 128  # 16 contraction tiles
NST = S // 128    # 8 sequence tiles
KVD = KVPC * D    # 256

# score block spans: for k-tile kt, valid q span is [128*kt, S), split into
# PSUM-bank-sized pieces aligned to 512 boundaries.
def _pieces(kt):
    qlo = 128 * kt
    out = []
    while qlo < S:
        qhi = min(S, (qlo // 512 + 1) * 512)
        out.append((qlo, qhi))
        qlo = qhi
    return out

# eS packing offset per kt (ragged)
ES_OFF = []
_o = 0
for _kt in range(NST):
    ES_OFF.append(_o)
    _o += S - 128 * _kt
ES_W = _o  # 4608


def build_kernel():
    nc = bacc.Bacc(None)
    xT = nc.dram_tensor("xT", [128, NKT * S], BF16, kind="ExternalInput")
    wq = nc.dram_tensor("wq", [128, HPC * NKT * D], BF16, kind="ExternalInput")
    wk = nc.dram_tensor("wk", [128, NKT * KVD], BF16, kind="ExternalInput")
    wv = nc.dram_tensor("wv", [128, NKT * KVD], BF16, kind="ExternalInput")
    wo = nc.dram_tensor("wo", [128, HPC * HID], BF16, kind="ExternalInput")
    cosT = nc.dram_tensor("cosT", [D, S], BF16, kind="ExternalInput")
    sinTs = nc.dram_tensor("sinTs", [D, S], BF16, kind="ExternalInput")
    p64 = nc.dram_tensor("p64", [D, D], BF16, kind="ExternalInput")
    dmask = nc.dram_tensor("dmask", [128, 128], BF16, kind="ExternalInput")
    out = nc.dram_tensor("out", [S, HID], F32, kind="ExternalOutput")

    with tile.TileContext(nc) as tc, ExitStack() as top:
        # ---- pools ------------------------------------------------------
        const = top.enter_context(tc.tile_pool(name="const", bufs=1))
        xk_pool = top.enter_context(tc.tile_pool(name="xk", bufs=1))
        wkv_pool = top.enter_context(tc.tile_pool(name="wkv", bufs=1))
        kT_pool = top.enter_context(tc.tile_pool(name="kT", bufs=1))
        v_pool = top.enter_context(tc.tile_pool(name="v", bufs=1))
        ctxT_pool = top.enter_context(tc.tile_pool(name="ctxT", bufs=1))
        wq_pool = top.enter_context(tc.tile_pool(name="wq", bufs=2))
        qT_pool = top.enter_context(tc.tile_pool(name="qT", bufs=2))
        eS_pool = top.enter_context(tc.tile_pool(name="eS", bufs=2))
        dn_pool = top.enter_context(tc.tile_pool(name="dn", bufs=2))
        rc_pool = top.enter_context(tc.tile_pool(name="rc", bufs=2))
        tmp_pool = top.enter_context(tc.tile_pool(name="tmp", bufs=2))
        wo_pool = top.enter_context(tc.tile_pool(name="wo", bufs=1))
        out_pool = top.enter_context(tc.tile_pool(name="outp", bufs=3))
        psQ = top.enter_context(tc.tile_pool(name="psQ", bufs=2, space="PSUM"))
        psS = top.enter_context(tc.tile_pool(name="psS", bufs=2, space="PSUM"))
        psC = top.enter_context(tc.tile_pool(name="psC", bufs=2, space="PSUM"))
        psD = top.enter_context(tc.tile_pool(name="psD", bufs=2, space="PSUM"))

        # ---- constants & input DMA --------------------------------------
        # Each triggering engine owns one HW DMA queue; spread the input
        # transfers so they run in parallel instead of serializing on sync.
        wk_sb = wkv_pool.tile([128, NKT * KVD], BF16, tag="wk")
        wv_sb = wkv_pool.tile([128, NKT * KVD], BF16, tag="wv")
        nc.scalar.dma_start(wk_sb[:], wk[:])
        nc.scalar.dma_start(wv_sb[:], wv[:])

        # x^T in 16 kt-chunks round-robined over 4 queues; projections
        # chase the DMA per chunk.
        xk = [xk_pool.tile([128, S], BF16, tag=f"xk{kt}", name=f"xk{kt}")
              for kt in range(NKT)]
        xk_eng = [nc.sync, nc.gpsimd]
        for kt in range(NKT):
            xk_eng[kt % 2].dma_start(xk[kt][:], xT[:, kt * S:(kt + 1) * S])

        cosT_sb = const.tile([D, S], BF16)
        sinTs_sb = const.tile([D, S], BF16)
        p64_sb = const.tile([D, D], BF16)
        dmask_sb = const.tile([128, 128], BF16)
        ones_bf = const.tile([128, 1], BF16)
        ones1p_f = const.tile([1, 128], F32)
        ones1p = const.tile([1, 128], F32R)
        nc.sync.dma_start(cosT_sb[:], cosT[:])
        nc.sync.dma_start(sinTs_sb[:], sinTs[:])
        nc.sync.dma_start(p64_sb[:], p64[:])
        nc.sync.dma_start(dmask_sb[:], dmask[:])
        nc.vector.memset(ones_bf[:], 1.0)
        nc.vector.memset(ones1p_f[:], 1.0)
        nc.vector.tensor_copy(ones1p[:], ones1p_f[:])

        wq_sb = [wq_pool.tile([128, NKT * D], BF16, tag="wqh",
                              name=f"wqh{h}") for h in range(HPC)]
        for h in range(2):
            nc.scalar.dma_start(wq_sb[h][:],
                                wq[:, h * NKT * D:(h + 1) * NKT * D])

        ctxT = [ctxT_pool.tile([D, S], BF16, tag=f"ctxT{h}", name=f"ctxT{h}")
                for h in range(HPC)]

        def rope_T(dst_sb, sc):
            """RoPE in T orientation on 512-col chunk sc of a [128, S] bf16
            tile, in place. P64 perm matmul + 3 DVE ops."""
            cs = slice(sc * 512, (sc + 1) * 512)
            rot = psD.tile([128, 512], F32, tag="ps_d")
            nc.tensor.matmul(rot[:], p64_sb[:], dst_sb[:, cs],
                             start=True, stop=True)
            tmp = tmp_pool.tile([128, 512], BF16, tag="ropetmp")
            nc.vector.tensor_mul(tmp[:], rot[:], sinTs_sb[:, cs])
            nc.vector.tensor_mul(dst_sb[:, cs], dst_sb[:, cs], cosT_sb[:, cs])
            nc.vector.tensor_add(dst_sb[:, cs], dst_sb[:, cs], tmp[:])

        # ---- K projection + rope: kT_c [128d, S] ------------------------
        kT = [kT_pool.tile([D, S], BF16, tag=f"kT{c}", name=f"kT{c}")
              for c in range(KVPC)]
        for c in range(KVPC):
            for sc in range(2):
                ps = psQ.tile([128, 512], F32, tag="ps_q")
                for kt in range(NKT):
                    nc.tensor.matmul(
                        ps[:], wk_sb[:, kt * KVD + c * D:kt * KVD + (c + 1) * D],
                        xk[kt][:, sc * 512:(sc + 1) * 512],
                        start=(kt == 0), stop=(kt == NKT - 1))
                nc.vector.tensor_copy(kT[c][:, sc * 512:(sc + 1) * 512], ps[:])
            for sc in range(2):
                rope_T(kT[c], sc)

        # ---- V projection (natural): v [128s, st*KVD + c*D + d] ---------
        v_sb = v_pool.tile([128, NST * KVD], BF16)
        for st in range(NST):
            ps = psQ.tile([128, 512], F32, tag="ps_q")
            for kt in range(NKT):
                nc.tensor.matmul(
                    ps[:, 0:KVD], xk[kt][:, st * 128:(st + 1) * 128],
                    wv_sb[:, kt * KVD:(kt + 1) * KVD],
                    start=(kt == 0), stop=(kt == NKT - 1))
            nc.vector.tensor_copy(v_sb[:, st * KVD:(st + 1) * KVD],
                                  ps[:, 0:KVD])

        # ---- per-head pipeline ------------------------------------------
        def q_proj(h):
            qT_h = qT_pool.tile([D, S], BF16, tag="qTh", name=f"qT{h}")
            for sc in range(2):
                ps = psQ.tile([128, 512], F32, tag="ps_q")
                for kt in range(NKT):
                    nc.tensor.matmul(
                        ps[:], wq_sb[h][:, kt * D:(kt + 1) * D],
                        xk[kt][:, sc * 512:(sc + 1) * 512],
                        start=(kt == 0), stop=(kt == NKT - 1))
                # PSUM->SBUF cast on the scalar engine (DVE is busier)
                nc.scalar.copy(qT_h[:, sc * 512:(sc + 1) * 512], ps[:])
            for sc in range(2):
                rope_T(qT_h, sc)
            return qT_h

        qT_cur = q_proj(0)

        wo_sb = None
        for h in range(HPC):
            c = h // (HPC // KVPC)  # local kv head
            # -- scores + exp (ragged blocks per kt) ----------------------
            eS = eS_pool.tile([128, ES_W], BF16, tag="eS", name=f"eS{h}")
            for kt in range(NST):
                for (qlo, qhi) in _pieces(kt):
                    w = qhi - qlo
                    ps = psS.tile([128, 512], F32, tag="ps_s")
                    nc.tensor.matmul(
                        ps[:, 0:w], kT[c][:, kt * 128:(kt + 1) * 128],
                        qT_cur[:, qlo:qhi], start=True, stop=True)
                    nc.scalar.activation(
                        eS[:, ES_OFF[kt] + qlo - 128 * kt:
                            ES_OFF[kt] + qhi - 128 * kt],
                        ps[:, 0:w],
                        mybir.ActivationFunctionType.Exp, scale=SCALE)
                # mask the diagonal 128x128 block
                nc.vector.tensor_mul(
                    eS[:, ES_OFF[kt]:ES_OFF[kt] + 128],
                    eS[:, ES_OFF[kt]:ES_OFF[kt] + 128], dmask_sb[:])

            # -- denominator ragged pre-sum, issued BEFORE the next head's
            # rope so it trails exp(h) while the PE runs Q-proj.  Short j=0
            # chain on the slow gpsimd engine, long j=1 chain on DVE.
            dnp = dn_pool.tile([128, S], BF16, tag="dnp", name=f"dnp{h}")
            for j, eng in ((0, nc.gpsimd), (1, nc.vector)):
                first = True
                for kt in range(NST):
                    qlo = 128 * kt
                    lo = max(qlo, j * 512)
                    hi = min(S, (j + 1) * 512)
                    if lo >= hi:
                        continue
                    src = eS[:, ES_OFF[kt] + lo - qlo:ES_OFF[kt] + hi - qlo]
                    if first:
                        eng.tensor_copy(dnp[:, lo:hi], src)
                        first = False
                    else:
                        eng.tensor_add(dnp[:, lo:hi], dnp[:, lo:hi], src)

            # -- next head's Q proj fills the PE while exp(h) runs --------
            if h + 1 < HPC:
                if h + 2 < HPC:
                    nc.gpsimd.dma_start(
                        wq_sb[h + 2][:],
                        wq[:, (h + 2) * NKT * D:(h + 3) * NKT * D])
                qT_nxt = q_proj(h + 1)

            # -- denominator ones-matmul + reciprocal ---------------------
            rc = rc_pool.tile([1, S], F32, tag="rc", name=f"rc{h}")
            rc_r = rc_pool.tile([1, S], F32R, tag="rcr", name=f"rcr{h}")
            dps = [None, None]
            for j in range(2):
                dps[j] = psD.tile([128, 512], F32, tag="ps_d", name=f"dps{j}")
                nc.tensor.matmul(dps[j][:1, 0:512], ones_bf[:],
                                 dnp[:, j * 512:(j + 1) * 512],
                                 start=True, stop=True)
            for j in range(2):
                nc.vector.reciprocal_approx_fast(
                    rc[:1, j * 512:(j + 1) * 512], dps[j][:1, 0:512])
                nc.vector.tensor_copy(rc_r[:1, j * 512:(j + 1) * 512],
                                      rc[:1, j * 512:(j + 1) * 512])

            # -- ctx matmul (ragged accumulate) + normalize ---------------
            pc = [None, None]
            for j in range(2):
                kts = [kt for kt in range(NST)
                       if max(128 * kt, j * 512) < (j + 1) * 512]
                pc[j] = psC.tile([128, 512], F32, tag="ps_c", name=f"pc{j}")
                for kt in kts:
                    qlo = 128 * kt
                    lo = max(qlo, j * 512)
                    hi = (j + 1) * 512
                    nc.tensor.matmul(
                        pc[j][:, lo - j * 512:hi - j * 512],
                        v_sb[:, kt * KVD + c * D:kt * KVD + (c + 1) * D],
                        eS[:, ES_OFF[kt] + lo - qlo:ES_OFF[kt] + hi - qlo],
                        start=(kt == kts[0]), stop=(kt == kts[-1]))
            for j in range(2):
                bc = psD.tile([128, 512], F32, tag="ps_d")
                nc.tensor.matmul(bc[:], ones1p[:],
                                 rc_r[:1, j * 512:(j + 1) * 512],
                                 start=True, stop=True)
                rb = tmp_pool.tile([128, 512], F32, tag="rbtmp")
                nc.scalar.copy(rb[:], bc[:])
                nc.vector.tensor_mul(
                    ctxT[h][:, j * 512:(j + 1) * 512], pc[j][:], rb[:])

            if h + 1 < HPC:
                qT_cur = qT_nxt
            if h == 3:  # wo arrives while attention still running
                wo_sb = wo_pool.tile([128, HPC * HID], BF16)
                nc.sync.dma_start(wo_sb[:], wo[:])

        # ---- O projection ----------------------------------------------
        for st in range(NST):
            for ec in range(HID // 512):
                if (st * 4 + ec) % 2:
                    po = psS.tile([128, 512], F32, tag="ps_s", name=f"po{st}_{ec}")
                else:
                    po = psQ.tile([128, 512], F32, tag="ps_q", name=f"po{st}_{ec}")
                for h in range(HPC):
                    nc.tensor.matmul(
                        po[:], ctxT[h][:, st * 128:(st + 1) * 128],
                        wo_sb[:, h * HID + ec * 512:h * HID + (ec + 1) * 512],
                        start=(h == 0), stop=(h == HPC - 1))
                ot = out_pool.tile([128, 512], F32, tag="ot")
                if (st * 4 + ec) % 2:
                    nc.scalar.copy(ot[:], po[:])
                    nc.scalar.dma_start(
                        out[st * 128:(st + 1) * 128,
                            ec * 512:(ec + 1) * 512], ot[:])
                else:
                    nc.vector.tensor_copy(ot[:], po[:])
                    nc.sync.dma_start(
                        out[st * 128:(st + 1) * 128,
                            ec * 512:(ec + 1) * 512], ot[:])
    nc.finalize()
    return nc


def host_prep(hidden_states, Wq, Wk, Wv, Wo):
    """Pre-transpose/cast/relayout all inputs on the host (bf16)."""
    bf = ml_dtypes.bfloat16
    xTs = []
    for b in range(B):
        t = hidden_states[b].T.reshape(NKT, 128, S).transpose(1, 0, 2)
        xTs.append(np.ascontiguousarray(t.astype(bf)).reshape(128, NKT * S))
    halves = []
    for hf in range(2):
        wqh = Wq[:, 1024 * hf:1024 * (hf + 1)].reshape(NKT, 128, HPC, D)
        wqh = np.ascontiguousarray(
            wqh.transpose(1, 2, 0, 3).astype(bf)).reshape(128, HPC * NKT * D)
        wkh = Wk[:, KVD * hf:KVD * (hf + 1)].reshape(NKT, 128, KVD)
        wkh = np.ascontiguousarray(
            wkh.transpose(1, 0, 2).astype(bf)).reshape(128, NKT * KVD)
        wvh = Wv[:, KVD * hf:KVD * (hf + 1)].reshape(NKT, 128, KVD)
        wvh = np.ascontiguousarray(
            wvh.transpose(1, 0, 2).astype(bf)).reshape(128, NKT * KVD)
        woh = Wo[1024 * hf:1024 * (hf + 1), :].reshape(HPC, 128, HID)
        woh = np.ascontiguousarray(
            woh.transpose(1, 0, 2).astype(bf)).reshape(128, HPC * HID)
        halves.append((wqh, wkh, wvh, woh))

    inv_freq = 1.0 / (10000.0 ** (np.arange(0, D, 2, dtype=np.float64) / D))
    t = np.arange(S, dtype=np.float64)
    freqs = np.outer(t, inv_freq)
    emb = np.concatenate([freqs, freqs], -1)
    cosT = np.ascontiguousarray(np.cos(emb).T).astype(bf)
    sinTs_f = np.sin(emb).T.copy()
    sinTs_f[:64] *= -1.0
    sinTs = np.ascontiguousarray(sinTs_f).astype(bf)
    p64 = np.zeros((D, D), dtype=np.float32)
    for d in range(D):
        p64[d, (d + 64) % D] = 1.0
    p64 = p64.astype(bf)
    dmask = np.triu(np.ones((128, 128), dtype=np.float32)).astype(bf)
    return xTs, halves, cosT, sinTs, p64, dmask


_CACHE = {}


def kernel(hidden_states, Wq, Wk, Wv, Wo, _trace=False, _tmpdir=None):
    hidden_states = np.ascontiguousarray(hidden_states, dtype=np.float32)
    Wq = np.ascontiguousarray(Wq, dtype=np.float32)
    Wk = np.ascontiguousarray(Wk, dtype=np.float32)
    Wv = np.ascontiguousarray(Wv, dtype=np.float32)
    Wo = np.ascontiguousarray(Wo, dtype=np.float32)

    if "nc" not in _CACHE:
        _CACHE["nc"] = build_kernel()
    nc = _CACHE["nc"]
    xTs, halves, cosT, sinTs, p64, dmask = host_prep(
        hidden_states, Wq, Wk, Wv, Wo)

    in_maps = []
    for cid in range(8):
        b, hf = cid // 2, cid % 2
        wqh, wkh, wvh, woh = halves[hf]
        in_maps.append({
            "xT": xTs[b], "wq": wqh, "wk": wkh, "wv": wvh, "wo": woh,
            "cosT": cosT, "sinTs": sinTs, "p64": p64, "dmask": dmask,
        })
    res = run_bass_kernel_spmd(nc, in_maps, list(range(8)),
                               trace=_trace, tmpdir=_tmpdir)
    out = np.zeros((B, S, HID), dtype=np.float32)
    for cid in range(8):
        out[cid // 2] += res.results[cid]["out"]
    if _trace:
        return out, res
    return out
